# revision 29
# baseline (speedup 1.0000x reference)
"""Multi-head attention (B=8, N=1024, C=768, H=8) on 8 Trainium2 NeuronCores.

Sharding: pure data-parallel over batch — core b computes batch element b
end-to-end (no collectives).

Per-core algorithm (working dtype fp16: full PE rate + fast weight load;
fp32 PSUM accumulation everywhere; softmax-sum chain kept in float32r):
  1. x arrives pre-transposed from the host: xT [C,N] as 6 [128,1024] tiles
     (plain contiguous DMA; no on-device transpose cost).
  2. qT/kT per head in padded [128,N] layout (zero weight columns pad head dim
     96->128 so the scores contraction uses K=128), V in natural [N,C] layout
     with a ones-column appended per head (softmax sums come out of the AV
     matmul for free).
  3. Scores+exp run ONE HEAD AHEAD of the AV consumption (head-lookahead
     software pipeline): head h's window does AV(h) interleaved with
     scores+exp(h+1), so ACT's exp latency never gates the PE, and the last
     head's window is pure AV. Head 0's scores+exp hide inside the V
     production phase. S^T per (head, j-tile) lands in one 2-bank PSUM tile
     [128,1024]; E^T = exp(S^T * hd^-0.5) is ONE [128,1024] ACT instruction
     (no max subtraction: |scores| <~ 6).
  4. O'^T[h] = sum_jt  V_aug[jt,h]-stationary @ E^T[jt]: PSUM [97, 512] x2,
     row 96 = softmax sums per i.
  5. Late normalization: sums row -> SBUF (DVE), broadcast 1/sums via K=1
     matmul outer(ones, sums), reciprocal on DVE, then partition-shifted
     DVE multiplies write the normalized AO^T directly into PACKED 96-dense
     [768, N] tiles (32-aligned partition blocks), so the projection
     contracts 6 packed k-tiles instead of 8 zero-padded ones.
  6. y = sum over 6 packed AO tiles @ packed proj_w: natural [N,C] -> DMA out
     per half so the tail eviction+DMA pipeline is short.

Engine split: PE matmuls; ACT exp only; DVE qk/V evictions + softmax
normalization chain; GPSIMD V ones-columns; SP DMA.

Bias handling: k-bias provably cancels in softmax; q-bias added at qT eviction
(per-partition); v-bias and proj-bias folded host-side (y += bv @ Wp + bp).
All biases are zero for this problem so those paths are skipped.
"""

import numpy as np

import concourse.bacc as bacc
import concourse.tile as tile
import concourse.mybir as mybir
from concourse import masks
from concourse.bass_utils import run_bass_kernel_spmd

f32 = mybir.dt.float32
f32r = mybir.dt.float32r
bf16 = mybir.dt.bfloat16
AF = mybir.ActivationFunctionType

import os
import ml_dtypes
WDT_MODE = os.environ.get("KERNEL_WDT", "fp16")
WDT = {"bf16": bf16, "fp16": mybir.dt.float16, "f32r": f32r}[WDT_MODE]
WNP = {"bf16": ml_dtypes.bfloat16, "fp16": np.float16, "f32r": np.float32}[WDT_MODE]

B, N, C = 8, 1024, 768
H, HD = 8, 96
NT, CT = N // 128, C // 128  # 8 token tiles, 6 channel tiles
PAD = 128                    # padded per-head dim for q/k
SCALE = float(HD) ** -0.5
VW = HD + 1                  # head block width in V buffer (96 v cols + ones)


def _blk(s):
    """Largest legal engine partition-access size starting at partition s."""
    return {0: 128, 32: 32, 64: 64, 96: 32}[s % 128]


def _pack_segments(h):
    """Split head h's 96 AO rows into (src_row, dst_tile, dst_row, n) chunks
    legal for partition-shifted DVE writes into the packed [768] layout."""
    segs = []
    a = 0
    while a < 96:
        g = 96 * h + a
        kd, r = g // 128, g % 128
        n = min(96 - a, _blk(a), _blk(r), 128 - r)
        segs.append((a, kd, r, n))
        a += n
    return segs


def _emit_body(nc, tc, pools, tensors, with_qbias, first=True):
    stage, wstage, epool, npool, ps = pools
    xt_d, wqk, wv, pw, qb, out = tensors["io"]
    ones_f, ones_row = tensors["const"]
    wv_sb, pw_sb, qb_sb = tensors["w"]
    xT, qkT, V_sb, AOpk = tensors["buf"]

    # head-0 q weights first so the first qk matmul is gated only by the
    # first xT tile; k weights (t=H) issued after xT so xT tiles stream in
    # as the t=0 matmuls consume them
    PROC = [1, 2, 3, 4, 5, 6, 7, 0]
    wt_pre = {}
    wt_pre[0] = wstage.tile([128, CT * PAD], WDT, tag="wqk", name="wtp0")
    nc.sync.dma_start(wt_pre[0][:], wqk[PROC[0]])

    # Phase A: xT tiles come pre-transposed from the host. Split across the
    # two DGE paths (HWDGE via SP, SWDGE via gpsimd) so per-DMA generator
    # overhead doesn't serialize the startup feed.
    for ct in (1, 3, 5):
        nc.gpsimd.dma_start(out=xT[ct][:], in_=xt_d[ct])
    for ct in (0, 2, 4):
        nc.sync.dma_start(xT[ct][:], xt_d[ct])

    wt_pre[H] = wstage.tile([128, CT * PAD], WDT, tag="wqk", name="wtp8")
    nc.sync.dma_start(wt_pre[H][:], wqk[H + PROC[0]])

    if first:
        for k in range(CT):
            nc.sync.dma_start(wv_sb[k][:], wv[k])
        if with_qbias:
            for h in range(H):
                nc.sync.dma_start(qb_sb[h][:], qb[h].rearrange("p -> p 1"))

    def emit_qk(t, wt=None, mid=None):
        """Produce qkT[t] (padded head tile) into the streaming ring.
        `mid` is emitted between the two ic halves (norm broadcast slot)."""
        qkT[t] = wstage.tile([128, N], WDT, tag="qkT", name=f"qkT{t}", bufs=6)
        if wt is None:
            wt = wstage.tile([128, CT * PAD], WDT, tag="wqk", name="wt")
            nc.sync.dma_start(wt[:], wqk[t])
        for ic in range(2):
            sl = slice(ic * 512, (ic + 1) * 512)
            pst = ps.tile([128, 512], f32, tag="qk", name="qkps", bufs=2)
            for k in range(CT):
                nc.tensor.matmul(
                    pst[:],
                    wt[:, k * PAD:(k + 1) * PAD],
                    xT[k][:, sl],
                    start=(k == 0), stop=(k == CT - 1),
                )
            if with_qbias and t < H:
                nc.scalar.activation(qkT[t][:, sl], pst[:], AF.Identity,
                                     bias=qb_sb[t][:])
            else:
                nc.scalar.copy(qkT[t][:, sl], pst[:])
            if ic == 0 and mid is not None:
                mid()

    et_map = {}
    early_yp = []

    def emit_se(h, jt):
        """Scores + exp for (head h, j-tile jt) -> et_map[h, jt]."""
        qt, kt = qkT[h], qkT[H + h]
        et = epool.tile([128, N], WDT, tag="et", name=f"et{h}_{jt}", bufs=20)
        sc = ps.tile([128, N], f32, tag="sc", name="sc", bufs=2)
        for ic in range(2):
            nc.tensor.matmul(
                sc[:, ic * 512:(ic + 1) * 512],
                kt[:, jt * 128:(jt + 1) * 128],
                qt[:, ic * 512:(ic + 1) * 512],
                start=True, stop=True,
            )
        nc.scalar.activation(et[:], sc[:], AF.Exp, scale=SCALE)
        et_map[h, jt] = et

    def emit_v_nt(nt):
        HB = C // 2  # 384 = 4 head blocks
        vv = V_sb[nt][:].rearrange("p (h s) -> p h s", h=H)
        for half in range(2):
            pv = ps.tile([128, HB], f32, tag="sc", name="vps", bufs=2)
            for k in range(CT):
                nc.tensor.matmul(pv[:], xT[k][:, nt * 128:(nt + 1) * 128],
                                 wv_sb[k][:, half * HB:(half + 1) * HB],
                                 start=(k == 0), stop=(k == CT - 1))
            nc.vector.tensor_copy(
                vv[:, half * 4:(half + 1) * 4, 0:HD],
                pv[:].rearrange("p (h d) -> p h d", h=4))
        for h in range(H):
            nc.gpsimd.tensor_copy(
                V_sb[nt][:, VW * h + HD: VW * h + VW], ones_f[:, 0:1])

    # heads processed in rotated order (PROC) so the LAST head (0) touches
    # only AOpk[0], minimizing the proj-start wait on its staging DMA

    def emit_norm_ic(p, ic, av, sums, aos):
        """One normalization half: broadcast 1/sums, then multiply — straight
        into the packed AO tiles when the partition blocks are legal (p%4 in
        {0,2}), else into staging (packed later by a shifted DMA)."""
        sl = slice(ic * 512, (ic + 1) * 512)
        nb = ps.tile([96, 512], f32, tag="sc", name="nb", bufs=2)
        nc.tensor.matmul(nb[:], ones_row[:], sums[ic][:], start=True, stop=True)
        rec = npool.tile([96, 512], f32, tag="rec", name="rec", bufs=3)
        nc.vector.reciprocal(rec[:], nb[:])
        if p % 4 in (0, 2):
            for (a, kd, r, n) in _pack_segments(p):
                nc.vector.tensor_mul(AOpk[kd][r:r + n, sl],
                                     av[ic][a:a + n, :], rec[a:a + n, :])
        else:
            nc.vector.tensor_mul(aos[0:96, sl], av[ic][0:96, :], rec[:])

    def emit_pack_dma(p, aos):
        """Partition-shifted SBUF->SBUF DMAs: staging rows -> packed AO."""
        if p % 4 in (0, 2):
            return
        a = 0
        while a < 96:
            g = 96 * p + a
            kd, r = g // 128, g % 128
            n = min(96 - a, 128 - r)
            nc.gpsimd.dma_start(out=AOpk[kd][r:r + n, :], in_=aos[a:a + n, :])
            a += n

    def window(i):
        """Window i: AV+norm for head PROC[i]; scores+exp per SE plan; q/k
        production for PROC[i+2] with norm broadcasts in its gaps."""
        p = PROC[i]
        se_main = ([(PROC[i + 1], jt) for jt in range(NT)] if i + 1 < 7 else
                   [] if i == 6 else
                   [(PROC[7], jt) for jt in (5, 6, 7)])
        # window 6's SE items run after its AV loop: they keep the PE busy
        # exactly while DVE copies the softmax sums
        se_post = [(PROC[7], jt) for jt in (2, 3, 4)] if i == 6 else []
        # windows 0..5 produce q/k for PROC[i+2]; window 5's tail also emits
        # the first two scores/exp of the last head (needs that q/k pair)
        se_tail = [(PROC[7], 0), (PROC[7], 1)] if i == 5 else []
        # the last window borrows the idle qk-production PSUM banks so its
        # AV never waits on the previous window's normalization reads
        av_tag = "qk" if i == len(PROC) - 1 else "av"
        av = [ps.tile([97, 512], f32, tag=av_tag, name=f"av{ic}", bufs=2)
              for ic in range(2)]
        for jt in range(NT):
            if jt < len(se_main):
                emit_se(*se_main[jt])
            vh = V_sb[jt][:, VW * p: VW * p + VW]
            et = et_map.pop((p, jt))
            for ic in range(2):
                nc.tensor.matmul(
                    av[ic][:],
                    vh,
                    et[:, ic * 512:(ic + 1) * 512],
                    start=(jt == 0), stop=(jt == NT - 1),
                )
        sums = []
        for ic in range(2):
            s = npool.tile([1, 512], f32r, tag="nrm", name="sums", bufs=4)
            if i == len(PROC) - 1:
                nc.scalar.copy(s[:], av[ic][96:97, :])
            else:
                nc.vector.tensor_copy(s[:], av[ic][96:97, :])
            sums.append(s)
        for it in se_post:
            emit_se(*it)
        aos = stage.tile([128, N], WDT, tag="aos", name="aos", bufs=2)
        if i + 2 < len(PROC):
            t = PROC[i + 2]
            emit_qk(t, mid=lambda: emit_norm_ic(p, 0, av, sums, aos))
            emit_norm_ic(p, 1, av, sums, aos)
            emit_qk(H + t)
        else:
            if i == len(PROC) - 1:
                HBp = C // 2
                for half in range(2):
                    yp = ps.tile([128, HBp], f32, tag="av", name="ypE", bufs=2)
                    for j, kd in enumerate([1, 2, 3, 4, 5]):
                        nc.tensor.matmul(yp[:], AOpk[kd][:, 0:128],
                                         pw_sb[kd][:, half * HBp:(half + 1) * HBp],
                                         start=(j == 0), stop=False)
                    early_yp.append(yp)
            emit_norm_ic(p, 0, av, sums, aos)
            emit_norm_ic(p, 1, av, sums, aos)
        emit_pack_dma(p, aos)
        for it in se_tail:
            emit_se(*it)
        if i == 0 and first:
            for j in range(CT):
                nc.sync.dma_start(pw_sb[j][:], pw[j])

    # prologue: q/k for the first two processed heads; V production with the
    # first head's scores+exp and the second q/k pair hidden under it
    emit_qk(PROC[0], wt_pre[0])
    emit_qk(H + PROC[0], wt_pre[H])
    for nt in range(NT):
        emit_v_nt(nt)
        emit_se(PROC[0], nt)
        if nt == 5:
            emit_qk(PROC[1])
        if nt == 6:
            emit_qk(H + PROC[1])
    for i in range(H):
        window(i)

    # ---- Phase E: output projection over 6 packed AO tiles ----
    HB = C // 2
    for it in range(NT):
        yst = stage.tile([128, C], f32, tag="ys", name="yst", bufs=2)
        tsl = slice(it * 128, (it + 1) * 128)
        for half in range(2):
            hsl = slice(half * HB, (half + 1) * HB)
            if it == 0:
                yp = early_yp[half]
                nc.tensor.matmul(yp[:], AOpk[0][:, tsl], pw_sb[0][:, hsl],
                                 start=False, stop=True)
            else:
                yp = ps.tile([128, HB], f32, tag="av", name="yps", bufs=2)
                # kd consumption ordered by AO-tile readiness (kd0 is written
                # by the last processed head, so it accumulates last)
                for j, kd in enumerate([1, 2, 3, 4, 5, 0]):
                    nc.tensor.matmul(yp[:], AOpk[kd][:, tsl], pw_sb[kd][:, hsl],
                                     start=(j == 0), stop=(j == CT - 1))
            if it == NT - 1 and half == 1:
                qs = [slice(half * HB, half * HB + HB // 2),
                      slice(half * HB + HB // 2, (half + 1) * HB)]
                nc.scalar.copy(yst[:, qs[0]], yp[:, 0:HB // 2])
                nc.vector.tensor_copy(yst[:, qs[1]], yp[:, HB // 2:HB])
                for q in qs:
                    nc.sync.dma_start(out[tsl, q], yst[:, q])
            else:
                if half == 0:
                    nc.scalar.copy(yst[:, hsl], yp[:])
                else:
                    nc.vector.tensor_copy(yst[:, hsl], yp[:])
                nc.sync.dma_start(out[tsl, hsl], yst[:, hsl])


def build_program(with_qbias=False, repeat=1):
    """Build + bacc-compile the single-core SPMD program."""
    nc = bacc.Bacc("TRN2", target_bir_lowering=False)
    xt_d = nc.dram_tensor("xt", [CT, 128, N], WDT, kind="ExternalInput")
    wqk = nc.dram_tensor("wqk", [2 * H, 128, CT * PAD], WDT, kind="ExternalInput")
    wv = nc.dram_tensor("wv", [CT, 128, C], WDT, kind="ExternalInput")
    pw = nc.dram_tensor("pw", [CT, 128, C], WDT, kind="ExternalInput")
    qb = (nc.dram_tensor("qb", [H, PAD], f32, kind="ExternalInput")
          if with_qbias else None)
    out = nc.dram_tensor("out", [N, C], f32, kind="ExternalOutput")

    with tile.TileContext(nc) as tc:
        with tc.tile_pool(name="const", bufs=1) as constp, \
             tc.tile_pool(name="persist", bufs=1) as persist, \
             tc.tile_pool(name="stage", bufs=3) as stage, \
             tc.tile_pool(name="wstage", bufs=4) as wstage, \
             tc.tile_pool(name="epool", bufs=6) as epool, \
             tc.tile_pool(name="npool", bufs=2) as npool, \
             tc.tile_pool(name="ps", bufs=2, space="PSUM") as ps:

            ones_f = constp.tile([128, HD], f32, tag="ones_f", name="ones_f")
            nc.vector.memset(ones_f[:], 1.0)
            ones_row = constp.tile([1, HD], f32r, tag="ones_row", name="ones_row")
            nc.vector.tensor_copy(ones_row[:], ones_f[0:1, :])

            wv_sb = [persist.tile([128, C], WDT, tag=f"wv{k}", name=f"wv{k}")
                     for k in range(CT)]
            pw_sb = [persist.tile([128, C], WDT, tag=f"pw{k}", name=f"pw{k}")
                     for k in range(CT)]
            qb_sb = None
            if with_qbias:
                qb_sb = [persist.tile([128, 1], f32, tag=f"qb{h}", name=f"qb{h}")
                         for h in range(H)]

            xT = [persist.tile([128, N], WDT, tag=f"xT{k}", name=f"xT{k}")
                  for k in range(CT)]
            qkT = {}
            V_sb = [persist.tile([128, VW * H], WDT, tag=f"V{nt}", name=f"V{nt}")
                    for nt in range(NT)]
            AOpk = [persist.tile([128, N], WDT, tag=f"AO{k}", name=f"AO{k}")
                    for k in range(CT)]

            pools = (stage, wstage, epool, npool, ps)
            tensors = {
                "io": (xt_d, wqk, wv, pw, qb, out),
                "const": (ones_f, ones_row),
                "w": (wv_sb, pw_sb, qb_sb),
                "buf": (xT, qkT, V_sb, AOpk),
            }
            for rep in range(repeat):
                _emit_body(nc, tc, pools, tensors, with_qbias, first=(rep == 0))

    nc.compile()
    return nc


def prepare_host_inputs(x, qkv_w, qkv_b, proj_w, proj_b):
    x = np.ascontiguousarray(np.asarray(x, dtype=np.float32))
    qkv_w = np.asarray(qkv_w, dtype=np.float32)
    qkv_b = np.asarray(qkv_b, dtype=np.float32)
    proj_w = np.asarray(proj_w, dtype=np.float32)
    proj_b = np.asarray(proj_b, dtype=np.float32)

    wq, wk, wv_np = qkv_w[:, 0:C], qkv_w[:, C:2 * C], qkv_w[:, 2 * C:3 * C]
    bq, bv = qkv_b[0:C], qkv_b[2 * C:3 * C]

    wqk_np = np.zeros((2 * H, CT, 128, PAD), WNP)
    for h in range(H):
        wqk_np[h, :, :, 0:HD] = wq[:, h * HD:(h + 1) * HD].reshape(CT, 128, HD)
        wqk_np[H + h, :, :, 0:HD] = wk[:, h * HD:(h + 1) * HD].reshape(CT, 128, HD)
    # [t, c-tile, c-in-tile, d] -> [t, c-in-tile, c-tile*d] so each per-t DMA
    # is one contiguous 128x768 block
    wqk_np = np.ascontiguousarray(
        wqk_np.transpose(0, 2, 1, 3).reshape(2 * H, 128, CT * PAD))
    wv_t = np.ascontiguousarray(wv_np.reshape(CT, 128, C)).astype(WNP)
    pw_t = np.ascontiguousarray(proj_w.reshape(CT, 128, C)).astype(WNP)

    with_qbias = bool(np.any(bq))
    base = {"wqk": wqk_np, "wv": wv_t, "pw": pw_t}
    if with_qbias:
        qb_np = np.zeros((H, PAD), np.float32)
        for h in range(H):
            qb_np[h, 0:HD] = bq[h * HD:(h + 1) * HD]
        base["qb"] = qb_np

    # v-bias and proj-bias commute past attention/proj -> host-side add
    post_add = bv @ proj_w + proj_b
    in_maps = [
        dict(base, xt=np.ascontiguousarray(x[b].T).astype(WNP).reshape(CT, 128, N))
        for b in range(B)
    ]
    return in_maps, with_qbias, post_add


def kernel(x, qkv_w, qkv_b, proj_w, proj_b):
    in_maps, with_qbias, post_add = prepare_host_inputs(
        x, qkv_w, qkv_b, proj_w, proj_b)
    nc = build_program(with_qbias=with_qbias)
    res = run_bass_kernel_spmd(nc, in_maps, core_ids=list(range(B)))
    y = np.stack([res.results[b]["out"] for b in range(B)], axis=0)
    if np.any(post_add):
        y = y + post_add[None, None, :].astype(np.float32)
    return np.ascontiguousarray(y.astype(np.float32))


# revision 35
# speedup vs baseline: 1.0214x; 1.0214x over previous
"""Multi-head attention (B=8, N=1024, C=768, H=8) on 8 Trainium2 NeuronCores.

Sharding: pure data-parallel over batch — core b computes batch element b
end-to-end (no collectives).

Per-core algorithm (working dtype fp16: full PE rate + fast weight load;
fp32 PSUM accumulation everywhere; softmax-sum chain kept in float32r):
  1. x arrives pre-transposed from the host: xT [C,N] as 6 [128,1024] tiles
     (plain contiguous DMA; no on-device transpose cost).
  2. qT/kT per head in padded [128,N] layout (zero weight columns pad head dim
     96->128 so the scores contraction uses K=128), V in natural [N,C] layout
     with a ones-column appended per head (softmax sums come out of the AV
     matmul for free).
  3. Scores+exp run ONE HEAD AHEAD of the AV consumption (head-lookahead
     software pipeline): head h's window does AV(h) interleaved with
     scores+exp(h+1), so ACT's exp latency never gates the PE, and the last
     head's window is pure AV. Head 0's scores+exp hide inside the V
     production phase. S^T per (head, j-tile) lands in one 2-bank PSUM tile
     [128,1024]; E^T = exp(S^T * hd^-0.5) is ONE [128,1024] ACT instruction
     (no max subtraction: |scores| <~ 6).
  4. O'^T[h] = sum_jt  V_aug[jt,h]-stationary @ E^T[jt]: PSUM [97, 512] x2,
     row 96 = softmax sums per i.
  5. Late normalization: sums row -> SBUF (DVE), broadcast 1/sums via K=1
     matmul outer(ones, sums), reciprocal on DVE, then partition-shifted
     DVE multiplies write the normalized AO^T directly into PACKED 96-dense
     [768, N] tiles (32-aligned partition blocks), so the projection
     contracts 6 packed k-tiles instead of 8 zero-padded ones.
  6. y = sum over 6 packed AO tiles @ packed proj_w: natural [N,C] -> DMA out
     per half so the tail eviction+DMA pipeline is short.

Engine split: PE matmuls; ACT exp only; DVE qk/V evictions + softmax
normalization chain; GPSIMD V ones-columns; SP DMA.

Bias handling: k-bias provably cancels in softmax; q-bias added at qT eviction
(per-partition); v-bias and proj-bias folded host-side (y += bv @ Wp + bp).
All biases are zero for this problem so those paths are skipped.
"""

import numpy as np

import concourse.bacc as bacc
import concourse.bass as bass
import concourse.tile as tile
import concourse.mybir as mybir
from concourse import masks
from concourse.bass_utils import run_bass_kernel_spmd

f32 = mybir.dt.float32
f32r = mybir.dt.float32r
bf16 = mybir.dt.bfloat16
AF = mybir.ActivationFunctionType

import os
import ml_dtypes
WDT_MODE = os.environ.get("KERNEL_WDT", "fp16")
WDT = {"bf16": bf16, "fp16": mybir.dt.float16, "f32r": f32r}[WDT_MODE]
WNP = {"bf16": ml_dtypes.bfloat16, "fp16": np.float16, "f32r": np.float32}[WDT_MODE]

B, N, C = 8, 1024, 768
H, HD = 8, 96
NT, CT = N // 128, C // 128  # 8 token tiles, 6 channel tiles
PAD = 128                    # padded per-head dim for q/k
SCALE = float(HD) ** -0.5
VW = HD + 1                  # head block width in V buffer (96 v cols + ones)


def _blk(s):
    """Largest legal engine partition-access size starting at partition s."""
    return {0: 128, 32: 32, 64: 64, 96: 32}[s % 128]


def _pack_segments(h):
    """Split head h's 96 AO rows into (src_row, dst_tile, dst_row, n) chunks
    legal for partition-shifted DVE writes into the packed [768] layout."""
    segs = []
    a = 0
    while a < 96:
        g = 96 * h + a
        kd, r = g // 128, g % 128
        n = min(96 - a, _blk(a), _blk(r), 128 - r)
        segs.append((a, kd, r, n))
        a += n
    return segs


def _emit_body(nc, tc, pools, tensors, with_qbias, first=True):
    stage, wstage, epool, npool, ps = pools
    xt_d, wqk, wv, pw, qb, out = tensors["io"]
    nsc_d = tensors["nsc"]
    ones_f, ones_row = tensors["const"]
    wv_sb, pw_sb, qb_sb = tensors["w"]
    xT, qkT, V_sb, AOpk = tensors["buf"]

    # head-0 q weights first so the first qk matmul is gated only by the
    # first xT tile; k weights (t=H) issued after xT so xT tiles stream in
    # as the t=0 matmuls consume them
    PROC = [1, 2, 3, 4, 5, 6, 7, 0]
    wt_pre = {}
    wt_pre[0] = wstage.tile([128, CT * PAD], WDT, tag="wqk", name="wtp0")
    nc.sync.dma_start(wt_pre[0][:], wqk[PROC[0]])

    # Phase A: xT tiles come pre-transposed from the host. Split across the
    # two DGE paths (HWDGE via SP, SWDGE via gpsimd) so per-DMA generator
    # overhead doesn't serialize the startup feed.
    for ct in (1, 3, 5):
        nc.gpsimd.dma_start(out=xT[ct][:], in_=xt_d[ct])
    for ct in (0, 2, 4):
        nc.sync.dma_start(xT[ct][:], xt_d[ct])

    wt_pre[H] = wstage.tile([128, CT * PAD], WDT, tag="wqk", name="wtp8")
    nc.sync.dma_start(wt_pre[H][:], wqk[H + PROC[0]])

    if first:
        for k in range(CT):
            nc.sync.dma_start(wv_sb[k][:], wv[k])
        if with_qbias:
            for h in range(H):
                nc.sync.dma_start(qb_sb[h][:], qb[h].rearrange("p -> p 1"))

    def emit_qk(t, wt=None, mid=None):
        """Produce qkT[t] (padded head tile) into the streaming ring.
        `mid` is emitted between the two ic halves (norm broadcast slot)."""
        qkT[t] = wstage.tile([128, N], WDT, tag="qkT", name=f"qkT{t}", bufs=6)
        if wt is None:
            wt = wstage.tile([128, CT * PAD], WDT, tag="wqk", name="wt")
            nc.sync.dma_start(wt[:], wqk[t])
        for ic in range(2):
            sl = slice(ic * 512, (ic + 1) * 512)
            pst = ps.tile([128, 512], f32, tag="qk", name="qkps", bufs=2)
            for k in range(CT):
                nc.tensor.matmul(
                    pst[:],
                    wt[:, k * PAD:(k + 1) * PAD],
                    xT[k][:, sl],
                    start=(k == 0), stop=(k == CT - 1),
                )
            if with_qbias and t < H:
                nc.scalar.activation(qkT[t][:, sl], pst[:], AF.Identity,
                                     bias=qb_sb[t][:])
            else:
                nc.scalar.copy(qkT[t][:, sl], pst[:])
            if ic == 0 and mid is not None:
                mid()

    et_map = {}
    early_yp = []
    pending_norm = []

    def emit_norm_fast(i, p, av):
        """Reciprocal row -> DRAM -> partition-broadcast back; av released
        after one unnormalized eviction. Multiplies deferred a window."""
        aou = stage.tile([128, N], WDT, tag="aou", name="aou", bufs=2)
        bcs = []
        for ic in range(2):
            sl = slice(ic * 512, (ic + 1) * 512)
            rrow = npool.tile([1, 512], WDT, tag="rr", name="rrow", bufs=4)
            with nc.allow_low_precision(reason="1/sums row in fp16; AO is fp16 anyway"):
                nc.vector.reciprocal(rrow[:], av[ic][96:97, :])
            nc.vector.tensor_copy(aou[0:96, sl], av[ic][0:96, :])
            slot = (2 * i + ic) % 12
            nc.gpsimd.dma_start(out=nsc_d[slot], in_=rrow[0:1, :])
            bc = npool.tile([96, 512], WDT, tag="bc", name="bc", bufs=4)
            dsl = nsc_d[slot]
            bap = bass.AP(tensor=dsl.tensor, offset=dsl.offset,
                          ap=[[0, 96]] + list(dsl.ap)[1:])
            nc.gpsimd.dma_start(out=bc[:], in_=bap)
            bcs.append(bc)
        pending_norm.append((p, aou, bcs))

    def flush_norm():
        if not pending_norm:
            return
        pp, aoup, bcsp = pending_norm.pop()
        aosn = (stage.tile([128, N], WDT, tag="aosf", name="aosf", bufs=2)
                if pp % 4 in (1, 3) else None)
        with nc.allow_low_precision(reason="fp16 normalization multiply; AO stored fp16"):
            for ic in range(2):
                sl = slice(ic * 512, (ic + 1) * 512)
                if pp % 4 in (0, 2):
                    for (a, kd, r, n) in _pack_segments(pp):
                        nc.vector.tensor_mul(AOpk[kd][r:r + n, sl],
                                             aoup[a:a + n, sl], bcsp[ic][a:a + n, :])
                else:
                    nc.vector.tensor_mul(aosn[0:96, sl], aoup[0:96, sl], bcsp[ic][:])
        if aosn is not None:
            emit_pack_dma(pp, aosn)

    def emit_se(h, jt):
        """Scores + exp for (head h, j-tile jt) -> et_map[h, jt]."""
        qt, kt = qkT[h], qkT[H + h]
        et = epool.tile([128, N], WDT, tag="et", name=f"et{h}_{jt}", bufs=20)
        sc = ps.tile([128, N], f32, tag="sc", name="sc", bufs=2)
        for ic in range(2):
            nc.tensor.matmul(
                sc[:, ic * 512:(ic + 1) * 512],
                kt[:, jt * 128:(jt + 1) * 128],
                qt[:, ic * 512:(ic + 1) * 512],
                start=True, stop=True,
            )
        nc.scalar.activation(et[:], sc[:], AF.Exp, scale=SCALE)
        et_map[h, jt] = et

    def emit_v_nt(nt):
        HB = C // 2  # 384 = 4 head blocks
        vv = V_sb[nt][:].rearrange("p (h s) -> p h s", h=H)
        for half in range(2):
            pv = ps.tile([128, HB], f32, tag="sc", name="vps", bufs=2)
            for k in range(CT):
                nc.tensor.matmul(pv[:], xT[k][:, nt * 128:(nt + 1) * 128],
                                 wv_sb[k][:, half * HB:(half + 1) * HB],
                                 start=(k == 0), stop=(k == CT - 1))
            nc.vector.tensor_copy(
                vv[:, half * 4:(half + 1) * 4, 0:HD],
                pv[:].rearrange("p (h d) -> p h d", h=4))
        for h in range(H):
            nc.gpsimd.tensor_copy(
                V_sb[nt][:, VW * h + HD: VW * h + VW], ones_f[:, 0:1])

    # heads processed in rotated order (PROC) so the LAST head (0) touches
    # only AOpk[0], minimizing the proj-start wait on its staging DMA

    def emit_norm_ic(p, ic, av, sums, aos):
        """One normalization half: broadcast 1/sums, then multiply — straight
        into the packed AO tiles when the partition blocks are legal (p%4 in
        {0,2}), else into staging (packed later by a shifted DMA)."""
        sl = slice(ic * 512, (ic + 1) * 512)
        nb = ps.tile([96, 512], f32, tag="sc", name="nb", bufs=2)
        nc.tensor.matmul(nb[:], ones_row[:], sums[ic][:], start=True, stop=True)
        rec = npool.tile([96, 512], f32, tag="rec", name="rec", bufs=3)
        nc.vector.reciprocal(rec[:], nb[:])
        if p % 4 in (0, 2):
            for (a, kd, r, n) in _pack_segments(p):
                nc.vector.tensor_mul(AOpk[kd][r:r + n, sl],
                                     av[ic][a:a + n, :], rec[a:a + n, :])
        else:
            nc.vector.tensor_mul(aos[0:96, sl], av[ic][0:96, :], rec[:])

    def emit_pack_dma(p, aos):
        """Partition-shifted SBUF->SBUF DMAs: staging rows -> packed AO."""
        if p % 4 in (0, 2):
            return
        a = 0
        while a < 96:
            g = 96 * p + a
            kd, r = g // 128, g % 128
            n = min(96 - a, 128 - r)
            nc.gpsimd.dma_start(out=AOpk[kd][r:r + n, :], in_=aos[a:a + n, :])
            a += n

    def window(i):
        """Window i: AV+norm for head PROC[i]; scores+exp per SE plan; q/k
        production for PROC[i+2] with norm broadcasts in its gaps."""
        p = PROC[i]
        se_main = ([(PROC[i + 1], jt) for jt in range(NT)] if i + 1 < 6 else
                   [(PROC[6], jt) for jt in range(2, NT)] if i == 5 else
                   [] if i == 6 else
                   [(PROC[7], jt) for jt in (5, 6, 7)])
        # window 6's SE items run after its AV loop: they keep the PE busy
        # exactly while DVE copies the softmax sums
        se_post = [(PROC[7], jt) for jt in (2, 3, 4)] if i == 6 else []
        # windows 0..5 produce q/k for PROC[i+2]; window 5's tail also emits
        # the first two scores/exp of the last head (needs that q/k pair)
        se_tail = ([(PROC[7], 0), (PROC[7], 1)] if i == 5 else
                   [(PROC[6], 0), (PROC[6], 1)] if i == 4 else [])
        # the last window borrows the idle qk-production PSUM banks so its
        # AV never waits on the previous window's normalization reads
        av_tag = "qk" if i == len(PROC) - 1 else "av"
        av = [ps.tile([97, 512], f32, tag=av_tag, name=f"av{ic}", bufs=2)
              for ic in range(2)]
        for jt in range(NT):
            if jt < len(se_main):
                emit_se(*se_main[jt])
            vh = V_sb[jt][:, VW * p: VW * p + VW]
            et = et_map.pop((p, jt))
            for ic in range(2):
                nc.tensor.matmul(
                    av[ic][:],
                    vh,
                    et[:, ic * 512:(ic + 1) * 512],
                    start=(jt == 0), stop=(jt == NT - 1),
                )
        if i + 2 < len(PROC):
            emit_norm_fast(i, p, av)
            flush_norm()
            t = PROC[i + 2]
            emit_qk(t)
            emit_qk(H + t)
        else:
            sums = []
            for ic in range(2):
                s = npool.tile([1, 512], f32r, tag="nrm", name="sums", bufs=4)
                nc.vector.tensor_copy(s[:], av[ic][96:97, :])
                sums.append(s)
            for it in se_post:
                emit_se(*it)
            aos = stage.tile([128, N], WDT, tag="aos", name="aos", bufs=2)
            flush_norm()
            if i == len(PROC) - 1:
                HBp = C // 2
                def _ype(half):
                    yp = ps.tile([128, HBp], f32, tag="av", name="ypE", bufs=2)
                    for j, kd in enumerate([1, 2, 3, 4, 5]):
                        nc.tensor.matmul(yp[:], AOpk[kd][:, 0:128],
                                         pw_sb[kd][:, half * HBp:(half + 1) * HBp],
                                         start=(j == 0), stop=False)
                    early_yp.append(yp)
                _ype(0)
                emit_norm_ic(p, 0, av, sums, aos)
                _ype(1)
                emit_norm_ic(p, 1, av, sums, aos)
            else:
                emit_norm_ic(p, 0, av, sums, aos)
                emit_norm_ic(p, 1, av, sums, aos)
            emit_pack_dma(p, aos)
        for it in se_tail:
            emit_se(*it)
        if i == 0 and first:
            for j in range(CT):
                nc.sync.dma_start(pw_sb[j][:], pw[j])

    # prologue: q/k for the first two processed heads; V production with the
    # first head's scores+exp and the second q/k pair hidden under it
    emit_qk(PROC[0], wt_pre[0])
    emit_qk(H + PROC[0], wt_pre[H])
    for nt in range(NT):
        emit_v_nt(nt)
        emit_se(PROC[0], nt)
        if nt == 5:
            emit_qk(PROC[1])
        if nt == 6:
            emit_qk(H + PROC[1])
    for i in range(H):
        window(i)

    # ---- Phase E: output projection over 6 packed AO tiles ----
    HB = C // 2
    for it in range(NT):
        yst = stage.tile([128, C], WDT, tag="ys", name="yst", bufs=2)
        tsl = slice(it * 128, (it + 1) * 128)
        for half in range(2):
            hsl = slice(half * HB, (half + 1) * HB)
            if it == 0:
                yp = early_yp[half]
                nc.tensor.matmul(yp[:], AOpk[0][:, tsl], pw_sb[0][:, hsl],
                                 start=False, stop=True)
            else:
                yp = ps.tile([128, HB], f32, tag="av", name="yps", bufs=2)
                # kd consumption ordered by AO-tile readiness (kd0 is written
                # by the last processed head, so it accumulates last)
                for j, kd in enumerate([1, 2, 3, 4, 5, 0]):
                    nc.tensor.matmul(yp[:], AOpk[kd][:, tsl], pw_sb[kd][:, hsl],
                                     start=(j == 0), stop=(j == CT - 1))
            if it == NT - 1 and half == 1:
                qs = [slice(half * HB, half * HB + HB // 2),
                      slice(half * HB + HB // 2, (half + 1) * HB)]
                nc.scalar.copy(yst[:, qs[0]], yp[:, 0:HB // 2])
                nc.vector.tensor_copy(yst[:, qs[1]], yp[:, HB // 2:HB])
                for q in qs:
                    nc.sync.dma_start(out[tsl, q], yst[:, q])
            else:
                if half == 0:
                    nc.scalar.copy(yst[:, hsl], yp[:])
                else:
                    nc.vector.tensor_copy(yst[:, hsl], yp[:])
                nc.sync.dma_start(out[tsl, hsl], yst[:, hsl])


def build_program(with_qbias=False, repeat=1):
    """Build + bacc-compile the single-core SPMD program."""
    nc = bacc.Bacc("TRN2", target_bir_lowering=False)
    xt_d = nc.dram_tensor("xt", [CT, 128, N], WDT, kind="ExternalInput")
    wqk = nc.dram_tensor("wqk", [2 * H, 128, CT * PAD], WDT, kind="ExternalInput")
    wv = nc.dram_tensor("wv", [CT, 128, C], WDT, kind="ExternalInput")
    pw = nc.dram_tensor("pw", [CT, 128, C], WDT, kind="ExternalInput")
    qb = (nc.dram_tensor("qb", [H, PAD], f32, kind="ExternalInput")
          if with_qbias else None)
    out = nc.dram_tensor("out", [N, C], WDT, kind="ExternalOutput")
    nsc = nc.dram_tensor("nsc", [12, 1, 512], WDT, kind="Internal")

    with tile.TileContext(nc) as tc:
        with tc.tile_pool(name="const", bufs=1) as constp, \
             tc.tile_pool(name="persist", bufs=1) as persist, \
             tc.tile_pool(name="stage", bufs=3) as stage, \
             tc.tile_pool(name="wstage", bufs=4) as wstage, \
             tc.tile_pool(name="epool", bufs=6) as epool, \
             tc.tile_pool(name="npool", bufs=2) as npool, \
             tc.tile_pool(name="ps", bufs=2, space="PSUM") as ps:

            ones_f = constp.tile([128, HD], f32, tag="ones_f", name="ones_f")
            nc.vector.memset(ones_f[:], 1.0)
            ones_row = constp.tile([1, HD], f32r, tag="ones_row", name="ones_row")
            nc.vector.tensor_copy(ones_row[:], ones_f[0:1, :])

            wv_sb = [persist.tile([128, C], WDT, tag=f"wv{k}", name=f"wv{k}")
                     for k in range(CT)]
            pw_sb = [persist.tile([128, C], WDT, tag=f"pw{k}", name=f"pw{k}")
                     for k in range(CT)]
            qb_sb = None
            if with_qbias:
                qb_sb = [persist.tile([128, 1], f32, tag=f"qb{h}", name=f"qb{h}")
                         for h in range(H)]

            xT = [persist.tile([128, N], WDT, tag=f"xT{k}", name=f"xT{k}")
                  for k in range(CT)]
            qkT = {}
            V_sb = [persist.tile([128, VW * H], WDT, tag=f"V{nt}", name=f"V{nt}")
                    for nt in range(NT)]
            AOpk = [persist.tile([128, N], WDT, tag=f"AO{k}", name=f"AO{k}")
                    for k in range(CT)]

            pools = (stage, wstage, epool, npool, ps)
            tensors = {
                "nsc": nsc,
                "io": (xt_d, wqk, wv, pw, qb, out),
                "const": (ones_f, ones_row),
                "w": (wv_sb, pw_sb, qb_sb),
                "buf": (xT, qkT, V_sb, AOpk),
            }
            for rep in range(repeat):
                _emit_body(nc, tc, pools, tensors, with_qbias, first=(rep == 0))

    nc.compile()
    return nc


def prepare_host_inputs(x, qkv_w, qkv_b, proj_w, proj_b):
    x = np.ascontiguousarray(np.asarray(x, dtype=np.float32))
    qkv_w = np.asarray(qkv_w, dtype=np.float32)
    qkv_b = np.asarray(qkv_b, dtype=np.float32)
    proj_w = np.asarray(proj_w, dtype=np.float32)
    proj_b = np.asarray(proj_b, dtype=np.float32)

    wq, wk, wv_np = qkv_w[:, 0:C], qkv_w[:, C:2 * C], qkv_w[:, 2 * C:3 * C]
    bq, bv = qkv_b[0:C], qkv_b[2 * C:3 * C]

    wqk_np = np.zeros((2 * H, CT, 128, PAD), WNP)
    for h in range(H):
        wqk_np[h, :, :, 0:HD] = wq[:, h * HD:(h + 1) * HD].reshape(CT, 128, HD)
        wqk_np[H + h, :, :, 0:HD] = wk[:, h * HD:(h + 1) * HD].reshape(CT, 128, HD)
    # [t, c-tile, c-in-tile, d] -> [t, c-in-tile, c-tile*d] so each per-t DMA
    # is one contiguous 128x768 block
    wqk_np = np.ascontiguousarray(
        wqk_np.transpose(0, 2, 1, 3).reshape(2 * H, 128, CT * PAD))
    wv_t = np.ascontiguousarray(wv_np.reshape(CT, 128, C)).astype(WNP)
    pw_t = np.ascontiguousarray(proj_w.reshape(CT, 128, C)).astype(WNP)

    with_qbias = bool(np.any(bq))
    base = {"wqk": wqk_np, "wv": wv_t, "pw": pw_t}
    if with_qbias:
        qb_np = np.zeros((H, PAD), np.float32)
        for h in range(H):
            qb_np[h, 0:HD] = bq[h * HD:(h + 1) * HD]
        base["qb"] = qb_np

    # v-bias and proj-bias commute past attention/proj -> host-side add
    post_add = bv @ proj_w + proj_b
    in_maps = [
        dict(base, xt=np.ascontiguousarray(x[b].T).astype(WNP).reshape(CT, 128, N))
        for b in range(B)
    ]
    return in_maps, with_qbias, post_add


def kernel(x, qkv_w, qkv_b, proj_w, proj_b):
    in_maps, with_qbias, post_add = prepare_host_inputs(
        x, qkv_w, qkv_b, proj_w, proj_b)
    nc = build_program(with_qbias=with_qbias)
    res = run_bass_kernel_spmd(nc, in_maps, core_ids=list(range(B)))
    y = np.stack([res.results[b]["out"] for b in range(B)], axis=0)
    if np.any(post_add):
        y = y + post_add[None, None, :].astype(np.float32)
    return np.ascontiguousarray(y.astype(np.float32))
